# revision 4
# baseline (speedup 1.0000x reference)
"""DynamicConvolution Trainium2 kernel (8 NeuronCores, data-parallel over batch).

Reference computation (per sample b):
  pooled = mean(x[b], spatial); h = relu(pooled @ w1 + b1)
  alpha  = softmax(h @ w2 + b2)                   [8]
  W[b]   = sum_k alpha[k] * kernels[k]            [256,256,3,3]
  y[b]   = conv2d(x[b], W[b], pad=1)              [256,56,56]

Sharding: batch 16 -> 2 samples per core; bank + MLP weights replicated.

v2 design (PE does conv only; mixing lives on the Vector engine):
  - bank arrives as [i128, (ot, it, k, s, o)] bf16: per (ot,it) block, 8
    contiguous k-slices [128, 1152] in conv-weight (s,o) order.  Kernel
    mixing = 8 chained scalar_tensor_tensor passes on VectorE
    (acc = bank_k * alpha[k] + acc, bf16 ping-pong, last pass writes wsb),
    each pass waiting only on its own k-slice DMA -> mixing chases the
    bank DMA and conv starts ~20us in instead of ~37us.
  - alpha broadcast [128,8]: diag(alpha) [8,8] on VectorE, then a
    ones[8,128]^T @ diag matmul -> every partition holds alpha[0..7].
  - DMA priority order (all inputs on the SP queue, in-order issue):
    x(b0) in 4-col-chunks per i-tile (attention reduces chase chunks),
    bank(o0,it0..1), bank(o1,it0..1), x(b1).  y writes + the [1,8]->[8,1]
    alpha transpose ride the idle GpSimd queue.
  - conv groups in DMA arrival order (b0,o0),(b0,o1),(b1,o0),(b1,o1):
    per (o_t, b, t) PSUM block of 18 accumulating matmuls
    [o128,448] += W[i128,o128]^T @ xpad[i128, 8x56 window]; ScalarE
    evacuates fp32; DMA out on GpSimd queue.

Sync discipline (walrus permits ONE semaphore wait per engine instruction):
  - conv matmul waits split: lhsT dep on InstLdweights (wsb via VectorE
    sem), rhs dep on InstMatmult; PSUM-reuse WAR rides InstMatmult, so
    each sample's x-chunk DMAs are pre-observed by a chained 1x1 "touch"
    matmul before the first conv matmul of the group.
  - mix stt ops: single external wait = own bank k-slice DMA; alpha
    scalar + acc are VectorE-local.
  - consts first observed on VectorE by a 1x1 copy; the diag op rides it.
  - all tiny MLP/diag PSUM tiles share one bank tag sequentially; the
    touch scratch lives in its own bank with PE-local WAW only.
"""

import numpy as np
import ml_dtypes
from contextlib import ExitStack

try:
    import concourse.bass as bass
except ImportError:  # fresh grading dir: repo paths not on sys.path yet
    import sys
    for p in ("/opt/trn_rl_repo", "/root/.axon_site/_ro/trn_rl_repo"):
        if p not in sys.path:
            sys.path.append(p)
    import concourse.bass as bass

import concourse.mybir as mybir
import concourse.tile as tile
from concourse import bacc
from concourse.tile import add_dep_helper
from concourse.bass_utils import run_bass_kernel_spmd

F32 = mybir.dt.float32
BF16 = mybir.dt.bfloat16
AX = mybir.AxisListType.X
RELU = mybir.ActivationFunctionType.Relu
EXP = mybir.ActivationFunctionType.Exp
COPY = mybir.ActivationFunctionType.Copy
MUL = mybir.AluOpType.mult
ADD = mybir.AluOpType.add

N_CORES = 8
B = 2               # samples per core
C = 256             # channels
IT = 2              # 128-channel input tiles
OT = 2              # 128-channel output tiles
H = W_IMG = 56
HW = H * W_IMG      # 3136
PADW = 58
PADHW = PADW * PADW  # 3364
NT = 7              # row blocks per image
TB = 448            # 8 rows x 56 cols per conv psum block
S = 9               # conv taps
WSL = S * 128       # 1152 = per (b,o_t,i_t) weight-slice elems
XCH = 4             # x DMA chunks per (b, i_tile)
XC = PADHW // XCH   # 841 elems per x chunk

# consts layout (fp32 [128, 344])
C_W1A, C_W1B, C_W2, C_B1, C_B2, C_ONES, C_ID8 = 0, 64, 128, 136, 200, 208, 336
C_COLS = 344

_cached = None


def _build():
    nc = bacc.Bacc()
    xin = nc.declare_dram_parameter("x", [B, C, PADHW], BF16, isOutput=False)
    bankin = nc.declare_dram_parameter("bank", [128, OT * IT * 8 * WSL], BF16,
                                       isOutput=False)
    cin = nc.declare_dram_parameter("consts", [128, C_COLS], F32, isOutput=False)
    bin_ = nc.declare_dram_parameter("bones", [8, 128], BF16, isOutput=False)
    y = nc.declare_dram_parameter("y", [B, C, HW], F32, isOutput=True)

    with tile.TileContext(nc) as tc, ExitStack() as ctx:
        sb = ctx.enter_context(tc.tile_pool(name="sb", bufs=1))
        conv_ps = ctx.enter_context(tc.tile_pool(name="cps", bufs=4, space="PSUM"))
        mlp_ps = ctx.enter_context(tc.tile_pool(name="mps", bufs=1, space="PSUM"))
        scr_ps = ctx.enter_context(tc.tile_pool(name="sps", bufs=1, space="PSUM"))

        xpad = sb.tile([128, B * IT * PADHW], BF16, tag="xpad")
        bank = sb.tile([128, OT * IT * 8 * WSL], BF16, tag="bank")
        wsb = sb.tile([128, B * OT * IT * WSL], BF16, tag="wsb")
        outsb = sb.tile([128, B * OT * NT * TB], F32, tag="outsb")
        consts = sb.tile([128, C_COLS], F32, tag="consts")
        bones = sb.tile([8, 128], BF16, tag="bones")
        acc0 = sb.tile([128, WSL], BF16, tag="acc0")
        acc1 = sb.tile([128, WSL], BF16, tag="acc1")
        acc = [acc0, acc1]
        alphab = sb.tile([128, B * 8], F32, tag="alphab")
        scratch = scr_ps.tile([1, 1], F32)

        def xv(b, it):
            base = (b * IT + it) * PADHW
            return xpad[:, base:base + PADHW].rearrange("p (r c) -> p r c", c=PADW)

        def pe_touch(ap):
            return nc.tensor.matmul(scratch[:], ap, ap, start=True, stop=True,
                                    skip_group_check=True)

        # ---------- input DMAs on the SP queue, in priority order ----------
        nc.sync.dma_start(consts[:], cin[:])
        nc.sync.dma_start(bones[:], bin_[:])

        def dma_x(b):
            for it in range(IT):
                base = (b * IT + it) * PADHW
                for cch in range(XCH):
                    nc.sync.dma_start(
                        xpad[:, base + cch * XC: base + (cch + 1) * XC],
                        xin[b, it * 128:(it + 1) * 128, cch * XC:(cch + 1) * XC])

        def dma_bank(ot):
            for it in range(IT):
                for k in range(8):
                    off = ((ot * IT + it) * 8 + k) * WSL
                    nc.sync.dma_start(bank[:, off:off + WSL],
                                      bankin[:, off:off + WSL])

        def touch_x(b):
            ts = []
            for it in range(IT):
                base = (b * IT + it) * PADHW
                for cch in range(XCH):
                    ts.append(pe_touch(xpad[0:1, base + cch * XC:
                                            base + cch * XC + 1]))
            return ts

        # ---------- shared small tiles
        ctch = sb.tile([1, 1], F32, tag="ctch")
        partials = sb.tile([128, B * IT * XCH], F32, tag="partials")
        psum2 = sb.tile([128, B * IT], F32, tag="psum2")
        pooled = sb.tile([128, B * IT], F32, tag="pooled")   # col (b, it)

        def attention(b):
            """pooled -> MLP -> softmax -> alphab[:, b*8:(b+1)*8] (f32)."""
            for it in range(IT):
                base = (b * IT + it) * PADHW
                for cch in range(XCH):
                    j = (b * IT + it) * XCH + cch
                    nc.vector.reduce_sum(
                        partials[:, j:j + 1],
                        xpad[:, base + cch * XC: base + (cch + 1) * XC], axis=AX)
            for it in range(IT):
                j = b * IT + it
                nc.vector.reduce_sum(psum2[:, j:j + 1],
                                     partials[:, j * XCH:(j + 1) * XCH], axis=AX)
                nc.vector.tensor_scalar_mul(pooled[:, j:j + 1],
                                            psum2[:, j:j + 1], 1.0 / HW)

            hT_ps = mlp_ps.tile([64, 1], F32, tag="mlp")
            nc.tensor.matmul(hT_ps[:], consts[0:1, C_B1:C_B1 + 64],
                             consts[0:1, C_ONES:C_ONES + 1],
                             start=True, stop=False)
            nc.tensor.matmul(hT_ps[:], consts[:, C_W1A:C_W1A + 64],
                             pooled[:, b * IT: b * IT + 1],
                             start=False, stop=False)
            nc.tensor.matmul(hT_ps[:], consts[:, C_W1B:C_W1B + 64],
                             pooled[:, b * IT + 1: b * IT + 2],
                             start=False, stop=True)
            hT = sb.tile([64, 1], F32, tag=f"hTs{b}")
            nc.scalar.activation(hT[:], hT_ps[:], RELU)

            sc_ps = mlp_ps.tile([1, 8], F32, tag="mlp")
            nc.tensor.matmul(sc_ps[:], consts[0:1, C_ONES:C_ONES + 1],
                             consts[0:1, C_B2:C_B2 + 8], start=True, stop=False)
            nc.tensor.matmul(sc_ps[:], hT[:], consts[0:64, C_W2:C_W2 + 8],
                             start=False, stop=True)

            scores = sb.tile([1, 8], F32, tag=f"scores{b}")
            nc.vector.tensor_copy(scores[:], sc_ps[:])
            mx = sb.tile([1, 1], F32, tag=f"mx{b}")
            nc.vector.reduce_max(mx[:], scores[:], axis=AX)
            subb = sb.tile([1, 8], F32, tag=f"subb{b}")
            nc.vector.tensor_scalar_sub(subb[:], scores[:], mx[:])
            ex = sb.tile([1, 8], F32, tag=f"ex{b}")
            nc.scalar.activation(ex[:], subb[:], EXP)
            z = sb.tile([1, 1], F32, tag=f"z{b}")
            nc.vector.reduce_sum(z[:], ex[:], axis=AX)
            rz = sb.tile([1, 1], F32, tag=f"rz{b}")
            nc.vector.reciprocal(rz[:], z[:])
            al = sb.tile([1, 8], F32, tag=f"al{b}")
            nc.vector.tensor_scalar_mul(al[:], ex[:], rz[:])

            a8 = sb.tile([8, 1], F32, tag=f"a8{b}")
            nc.gpsimd.dma_start(a8[:], al[:])          # [1,8] -> [8,1]
            diag8 = sb.tile([8, 8], BF16, tag=f"dg{b}")
            dg_i = nc.vector.tensor_scalar_mul(
                diag8[:], consts[0:8, C_ID8:C_ID8 + 8], a8[:, 0:1])
            if b == 0:
                ctch_i = nc.vector.tensor_copy(ctch[:], consts[0:1, 0:1])
                add_dep_helper(dg_i.ins, ctch_i.ins, sync=False,
                               reason="consts seen on DVE")
            aT_ps = mlp_ps.tile([128, 8], F32, tag="mlp")
            nc.tensor.matmul(aT_ps[:], bones[:], diag8[:], start=True, stop=True)
            nc.vector.tensor_copy(alphab[:, b * 8:(b + 1) * 8], aT_ps[:])

        def mix(b, ot, it):
            """wsb slice (b,ot,it) = sum_k alpha[b,k] * bank(ot,it,k) on DVE."""
            woff = ((b * OT + ot) * IT + it) * WSL
            blk = (ot * IT + it) * 8
            a = alphab[:, b * 8:(b + 1) * 8]
            nc.vector.tensor_scalar_mul(
                acc[0][:], bank[:, blk * WSL:(blk + 1) * WSL], a[:, 0:1])
            for k in range(1, 8):
                out = wsb[:, woff:woff + WSL] if k == 7 else acc[k % 2][:]
                nc.vector.scalar_tensor_tensor(
                    out, bank[:, (blk + k) * WSL:(blk + k + 1) * WSL],
                    a[:, k:k + 1], acc[1 - k % 2][:], MUL, ADD)

        def conv(b, ot, xtouch_last):
            gi = b * OT + ot
            for t in range(NT):
                ps = conv_ps.tile([128, TB], F32, tag="convps")
                n_mm = 0
                for it in range(IT):
                    woff = ((b * OT + ot) * IT + it) * WSL
                    v = xv(b, it)
                    for s in range(S):
                        kh, kw = s // 3, s % 3
                        mm = nc.tensor.matmul(
                            ps[:],
                            wsb[:, woff + s * 128: woff + (s + 1) * 128],
                            v[:, 8 * t + kh: 8 * t + kh + 8, kw:kw + 56],
                            start=(n_mm == 0), stop=(n_mm == 17))
                        if n_mm == 0 and t == 0 and xtouch_last is not None:
                            add_dep_helper(mm.ins, xtouch_last.ins, sync=False,
                                           reason="xpad observed")
                        n_mm += 1
                blk = gi * NT + t
                nc.scalar.activation(outsb[:, blk * TB:(blk + 1) * TB],
                                     ps[:], COPY)
                nc.gpsimd.dma_start(
                    y[b, ot * 128:(ot + 1) * 128, t * TB:(t + 1) * TB],
                    outsb[:, blk * TB:(blk + 1) * TB])

        def chain(ts, prev=None):
            if prev is not None and ts:
                add_dep_helper(ts[0].ins, prev.ins, sync=False, reason="chain")
            for t1, t0 in zip(ts[1:], ts[:-1]):
                add_dep_helper(t1.ins, t0.ins, sync=False, reason="chain")
            return ts

        # ---------- emission in intended runtime order ----------
        dma_x(0)
        dma_bank(0)
        dma_bank(1)
        dma_x(1)

        xt0 = chain(touch_x(0))
        attention(0)
        mix(0, 0, 0)
        mix(0, 0, 1)
        conv(0, 0, xt0[-1])
        mix(0, 1, 0)
        mix(0, 1, 1)
        conv(0, 1, None)

        xt1 = chain(touch_x(1), prev=xt0[-1])
        attention(1)
        mix(1, 0, 0)
        mix(1, 0, 1)
        conv(1, 0, xt1[-1])
        mix(1, 1, 0)
        mix(1, 1, 1)
        conv(1, 1, None)

    nc.compile()
    return nc


def _prep(x, kernels, w1, b1, w2, b2):
    """Host-side marshaling: dtype casts + layout/padding rearrangement only."""
    xp = np.zeros((16, C, PADW, PADW), dtype=ml_dtypes.bfloat16)
    xp[:, :, 1:57, 1:57] = x
    xh = np.ascontiguousarray(xp.reshape(16, C, PADHW))
    # bank[i, ((ot*IT+it)*8+k)*WSL + s*128 + o] = kernels[k, ot*128+o, it*128+i, s]
    kr = kernels.reshape(8, OT, 128, IT, 128, S)   # k, ot, o, it, i, s
    bankh = np.ascontiguousarray(
        kr.transpose(4, 1, 3, 0, 5, 2).reshape(128, OT * IT * 8 * WSL)
        .astype(ml_dtypes.bfloat16))
    consts = np.zeros((128, C_COLS), dtype=np.float32)
    consts[:, C_W1A:C_W1A + 64] = w1[0:128]
    consts[:, C_W1B:C_W1B + 64] = w1[128:256]
    consts[0:64, C_W2:C_W2 + 8] = w2
    consts[0, C_B1:C_B1 + 64] = b1
    consts[0, C_B2:C_B2 + 8] = b2
    consts[0, C_ONES:C_ONES + 128] = 1.0
    consts[0:8, C_ID8:C_ID8 + 8] = np.eye(8, dtype=np.float32)
    bones = np.ones((8, 128), dtype=ml_dtypes.bfloat16)
    return xh, bankh, consts, bones


def kernel(x, kernels, w1, b1, w2, b2):
    global _cached
    if _cached is None:
        _cached = _build()
    nc = _cached
    xh, bankh, consts, bones = _prep(np.asarray(x, dtype=np.float32),
                                     np.asarray(kernels, dtype=np.float32),
                                     np.asarray(w1, dtype=np.float32),
                                     np.asarray(b1, dtype=np.float32),
                                     np.asarray(w2, dtype=np.float32),
                                     np.asarray(b2, dtype=np.float32))
    in_maps = [{"x": xh[c * B:(c + 1) * B], "bank": bankh,
                "consts": consts, "bones": bones} for c in range(N_CORES)]
    res = run_bass_kernel_spmd(nc, in_maps, list(range(N_CORES)))
    out = np.concatenate(
        [res.results[c]["y"].reshape(B, C, H, W_IMG) for c in range(N_CORES)],
        axis=0)
    return out.astype(np.float32)


# revision 5
# speedup vs baseline: 1.0752x; 1.0752x over previous
"""DynamicConvolution Trainium2 kernel (8 NeuronCores, data-parallel over batch).

Reference computation (per sample b):
  pooled = mean(x[b], spatial); h = relu(pooled @ w1 + b1)
  alpha  = softmax(h @ w2 + b2)                   [8]
  W[b]   = sum_k alpha[k] * kernels[k]            [256,256,3,3]
  y[b]   = conv2d(x[b], W[b], pad=1)              [256,56,56]

Sharding: batch 16 -> 2 samples per core; bank + MLP weights replicated.

v3 design (PE does conv only; mixing on VectorE; alpha path stays on-chip):
  - bank arrives as [i128, (ot, it, k, s, o)] bf16: per (ot,it) block, 8
    contiguous k-slices [128, 1152] in conv-weight (s,o) order.  Kernel
    mixing = 8 chained scalar_tensor_tensor passes on VectorE
    (acc = bank_k * e_k + acc, bf16 ping-pong, last pass writes wsb),
    each pass waiting only on its own k-slice DMA.
  - softmax restructured so nothing round-trips through DRAM: e = exp(
    h@w2 + b2) unnormalized; [1,8]->[8,1] turns via tiny PE matmuls
    (lhsT=e, rhs=ones); the 1/sum(e) normalizer is folded into the conv
    PSUM evacuation as a per-sample activation scale.  1/HW and b1 fold
    into the relu activation (scale+bias).
  - alpha broadcast [128,8]: diag(e) [8,8] on VectorE, then a
    ones[8,128]^T @ diag matmul -> every partition holds e[0..7].
  - DMA priority order (inputs on the SP queue, in-order issue): x(b0)
    in 4-col-chunks per i-tile (attention reduces chase chunks),
    bank(o0), bank(o1,it0), bank(o1,it1), x(b1).  y writes ride the
    otherwise-idle GpSimd queue.
  - conv groups in DMA arrival order (b0,o0),(b0,o1),(b1,o0),(b1,o1):
    per (o_t, b, t) PSUM block of 18 accumulating matmuls
    [o128,448] += W[i128,o128]^T @ xpad[i128, 8x56 window]; 6 PSUM bufs
    let the scheduler run it0-tap matmuls of many blocks while it1 is
    still mixing; ScalarE evacuates fp32 (scaled by 1/z); DMA out.

Sync discipline (walrus permits ONE semaphore wait per engine instruction):
  - conv matmul waits split: lhsT dep on InstLdweights (wsb via VectorE
    sem), rhs dep on InstMatmult; PSUM-reuse WAR rides InstMatmult, so
    each sample's x-chunk DMAs are pre-observed by a chained 1x1 "touch"
    matmul before the first conv matmul of the group.
  - mix stt ops: single external wait = own bank k-slice DMA; alpha
    scalar + acc are VectorE-local.
  - consts first observed on VectorE by a 1x1 copy; the diag op rides it.
    rzsb first observed on ScalarE by a 1x1 copy; evacs ride it.
  - all tiny MLP PSUM tiles share one bank tag sequentially; the touch
    scratch lives in its own bank with PE-local WAW only.
"""

import numpy as np
import ml_dtypes
from contextlib import ExitStack

try:
    import concourse.bass as bass
except ImportError:  # fresh grading dir: repo paths not on sys.path yet
    import sys
    for p in ("/opt/trn_rl_repo", "/root/.axon_site/_ro/trn_rl_repo"):
        if p not in sys.path:
            sys.path.append(p)
    import concourse.bass as bass

import concourse.mybir as mybir
import concourse.tile as tile
from concourse import bacc
from concourse.tile import add_dep_helper
from concourse.bass_utils import run_bass_kernel_spmd

F32 = mybir.dt.float32
BF16 = mybir.dt.bfloat16
AX = mybir.AxisListType.X
RELU = mybir.ActivationFunctionType.Relu
EXP = mybir.ActivationFunctionType.Exp
COPY = mybir.ActivationFunctionType.Copy
MUL = mybir.AluOpType.mult
ADD = mybir.AluOpType.add

N_CORES = 8
B = 2               # samples per core
C = 256             # channels
IT = 2              # 128-channel input tiles
OT = 2              # 128-channel output tiles
H = W_IMG = 56
HW = H * W_IMG      # 3136
PADW = 58
PADHW = PADW * PADW  # 3364
NT = 7              # row blocks per image
TB = 448            # 8 rows x 56 cols per conv psum block
S = 9               # conv taps
WSL = S * 128       # 1152 = per (b,o_t,i_t) weight-slice elems
XCH = 4             # x DMA chunks per (b, i_tile)
XC = PADHW // XCH   # 841 elems per x chunk

# consts layout (fp32 [128, 288])
C_W1A, C_W1B, C_W2, C_B1C, C_B2, C_ONES, C_ID8 = 0, 64, 128, 136, 140, 148, 280
C_COLS = 288

_cached = None


def _build():
    nc = bacc.Bacc()
    xin = nc.declare_dram_parameter("x", [B, C, PADHW], BF16, isOutput=False)
    bankin = nc.declare_dram_parameter("bank", [128, OT * IT * 8 * WSL], BF16,
                                       isOutput=False)
    cin = nc.declare_dram_parameter("consts", [128, C_COLS], F32, isOutput=False)
    bin_ = nc.declare_dram_parameter("bones", [8, 128], BF16, isOutput=False)
    y = nc.declare_dram_parameter("y", [B, C, HW], F32, isOutput=True)

    with tile.TileContext(nc) as tc, ExitStack() as ctx:
        sb = ctx.enter_context(tc.tile_pool(name="sb", bufs=1))
        conv_ps = ctx.enter_context(tc.tile_pool(name="cps", bufs=6, space="PSUM"))
        mlp_ps = ctx.enter_context(tc.tile_pool(name="mps", bufs=1, space="PSUM"))
        scr_ps = ctx.enter_context(tc.tile_pool(name="sps", bufs=1, space="PSUM"))

        xpad = sb.tile([128, B * IT * PADHW], BF16, tag="xpad")
        bank = sb.tile([128, OT * IT * 8 * WSL], BF16, tag="bank")
        wsb = sb.tile([128, B * OT * IT * WSL], BF16, tag="wsb")
        outsb = sb.tile([128, B * OT * NT * TB], F32, tag="outsb")
        consts = sb.tile([128, C_COLS], F32, tag="consts")
        bones = sb.tile([8, 128], BF16, tag="bones")
        acc0 = sb.tile([128, WSL], BF16, tag="acc0")
        acc1 = sb.tile([128, WSL], BF16, tag="acc1")
        acc = [acc0, acc1]
        alphab = sb.tile([128, B * 8], F32, tag="alphab")
        rzsb = sb.tile([128, B], F32, tag="rzsb")
        scratch = scr_ps.tile([1, 1], F32)

        def xv(b, it):
            base = (b * IT + it) * PADHW
            return xpad[:, base:base + PADHW].rearrange("p (r c) -> p r c", c=PADW)

        def pe_touch(ap):
            return nc.tensor.matmul(scratch[:], ap, ap, start=True, stop=True,
                                    skip_group_check=True)

        # ---------- input DMAs on the SP queue, in priority order ----------
        nc.sync.dma_start(consts[:], cin[:])
        nc.sync.dma_start(bones[:], bin_[:])

        def dma_x(b):
            for it in range(IT):
                base = (b * IT + it) * PADHW
                for cch in range(XCH):
                    nc.sync.dma_start(
                        xpad[:, base + cch * XC: base + (cch + 1) * XC],
                        xin[b, it * 128:(it + 1) * 128, cch * XC:(cch + 1) * XC])

        def dma_bank(ot, its):
            for it in its:
                for k in range(8):
                    off = ((ot * IT + it) * 8 + k) * WSL
                    nc.sync.dma_start(bank[:, off:off + WSL],
                                      bankin[:, off:off + WSL])

        def touch_x(b):
            ts = []
            for it in range(IT):
                base = (b * IT + it) * PADHW
                for cch in range(XCH):
                    ts.append(pe_touch(xpad[0:1, base + cch * XC:
                                            base + cch * XC + 1]))
            return ts

        # ---------- shared small tiles
        ctch = sb.tile([1, 1], F32, tag="ctch")
        rztch = sb.tile([1, 1], F32, tag="rztch")
        partials = sb.tile([128, B * IT * XCH], F32, tag="partials")
        psum2 = sb.tile([128, B * IT], F32, tag="psum2")

        def attention(b):
            """channel sums -> MLP -> e=exp(scores) -> alphab, rzsb."""
            for it in range(IT):
                base = (b * IT + it) * PADHW
                for cch in range(XCH):
                    j = (b * IT + it) * XCH + cch
                    nc.vector.reduce_sum(
                        partials[:, j:j + 1],
                        xpad[:, base + cch * XC: base + (cch + 1) * XC], axis=AX)
            for it in range(IT):
                j = b * IT + it
                nc.vector.reduce_sum(psum2[:, j:j + 1],
                                     partials[:, j * XCH:(j + 1) * XCH], axis=AX)

            # h = relu((w1^T xsum)/HW + b1): scale+bias fold into the relu
            hT_ps = mlp_ps.tile([64, 1], F32, tag="mlp")
            nc.tensor.matmul(hT_ps[:], consts[:, C_W1A:C_W1A + 64],
                             psum2[:, b * IT: b * IT + 1],
                             start=True, stop=False)
            nc.tensor.matmul(hT_ps[:], consts[:, C_W1B:C_W1B + 64],
                             psum2[:, b * IT + 1: b * IT + 2],
                             start=False, stop=True)
            hT = sb.tile([64, 1], F32, tag=f"hTs{b}")
            nc.scalar.activation(hT[:], hT_ps[:], RELU,
                                 bias=consts[0:64, C_B1C:C_B1C + 1],
                                 scale=1.0 / HW)

            sc_ps = mlp_ps.tile([1, 8], F32, tag="mlp")
            nc.tensor.matmul(sc_ps[:], consts[0:1, C_ONES:C_ONES + 1],
                             consts[0:1, C_B2:C_B2 + 8], start=True, stop=False)
            nc.tensor.matmul(sc_ps[:], hT[:], consts[0:64, C_W2:C_W2 + 8],
                             start=False, stop=True)

            # e = exp(scores); scores ~ +-0.1 so no max-subtraction needed
            ex = sb.tile([1, 8], F32, tag=f"ex{b}")
            nc.scalar.activation(ex[:], sc_ps[:], EXP)
            z = sb.tile([1, 1], F32, tag=f"z{b}")
            nc.vector.reduce_sum(z[:], ex[:], axis=AX)
            rz = sb.tile([1, 1], F32, tag=f"rz{b}")
            nc.vector.reciprocal(rz[:], z[:])

            # rzsb[:, b] = 1/z broadcast to 128 partitions (for evac scale)
            rz_ps = mlp_ps.tile([128, 1], F32, tag="mlp")
            nc.tensor.matmul(rz_ps[:], consts[0:1, C_ONES:C_ONES + 128],
                             rz[:], start=True, stop=True)
            nc.vector.tensor_copy(rzsb[:, b:b + 1], rz_ps[:])
            rzg = nc.scalar.activation(rztch[:], rzsb[0:1, b:b + 1], COPY)

            # e as a column via PE transpose, then diag(e) and broadcast
            e_ps = mlp_ps.tile([8, 1], F32, tag="mlp")
            nc.tensor.matmul(e_ps[:], ex[:], consts[0:1, C_ONES:C_ONES + 1],
                             start=True, stop=True)
            e8 = sb.tile([8, 1], F32, tag=f"e8{b}")
            nc.vector.tensor_copy(e8[:], e_ps[:])
            diag8 = sb.tile([8, 8], BF16, tag=f"dg{b}")
            dg_i = nc.vector.tensor_scalar_mul(
                diag8[:], consts[0:8, C_ID8:C_ID8 + 8], e8[:, 0:1])
            if b == 0:
                ctch_i = nc.vector.tensor_copy(ctch[:], consts[0:1, 0:1])
                add_dep_helper(dg_i.ins, ctch_i.ins, sync=False,
                               reason="consts seen on DVE")
            aT_ps = mlp_ps.tile([128, 8], F32, tag="mlp")
            nc.tensor.matmul(aT_ps[:], bones[:], diag8[:], start=True, stop=True)
            nc.vector.tensor_copy(alphab[:, b * 8:(b + 1) * 8], aT_ps[:])
            return rzg

        def mix(b, ot, it):
            """wsb slice (b,ot,it) = sum_k e[b,k] * bank(ot,it,k) on DVE."""
            woff = ((b * OT + ot) * IT + it) * WSL
            blk = (ot * IT + it) * 8
            a = alphab[:, b * 8:(b + 1) * 8]
            nc.vector.tensor_scalar_mul(
                acc[0][:], bank[:, blk * WSL:(blk + 1) * WSL], a[:, 0:1])
            for k in range(1, 8):
                out = wsb[:, woff:woff + WSL] if k == 7 else acc[k % 2][:]
                nc.vector.scalar_tensor_tensor(
                    out, bank[:, (blk + k) * WSL:(blk + k + 1) * WSL],
                    a[:, k:k + 1], acc[1 - k % 2][:], MUL, ADD)

        def conv(b, ot, xtouch_last, rzgate):
            gi = b * OT + ot
            for t in range(NT):
                ps = conv_ps.tile([128, TB], F32, tag="convps")
                n_mm = 0
                for it in range(IT):
                    woff = ((b * OT + ot) * IT + it) * WSL
                    v = xv(b, it)
                    for s in range(S):
                        kh, kw = s // 3, s % 3
                        mm = nc.tensor.matmul(
                            ps[:],
                            wsb[:, woff + s * 128: woff + (s + 1) * 128],
                            v[:, 8 * t + kh: 8 * t + kh + 8, kw:kw + 56],
                            start=(n_mm == 0), stop=(n_mm == 17))
                        if n_mm == 0 and t == 0 and xtouch_last is not None:
                            add_dep_helper(mm.ins, xtouch_last.ins, sync=False,
                                           reason="xpad observed")
                        n_mm += 1
                blk = gi * NT + t
                ev = nc.scalar.activation(outsb[:, blk * TB:(blk + 1) * TB],
                                          ps[:], COPY,
                                          scale=rzsb[:, b:b + 1])
                if t == 0:
                    add_dep_helper(ev.ins, rzgate.ins, sync=False,
                                   reason="rzsb seen on ACT")
                nc.gpsimd.dma_start(
                    y[b, ot * 128:(ot + 1) * 128, t * TB:(t + 1) * TB],
                    outsb[:, blk * TB:(blk + 1) * TB])

        def chain(ts, prev=None):
            if prev is not None and ts:
                add_dep_helper(ts[0].ins, prev.ins, sync=False, reason="chain")
            for t1, t0 in zip(ts[1:], ts[:-1]):
                add_dep_helper(t1.ins, t0.ins, sync=False, reason="chain")
            return ts

        # ---------- emission in intended runtime order ----------
        dma_x(0)
        dma_bank(0, (0, 1))
        dma_bank(1, (0,))
        dma_bank(1, (1,))
        dma_x(1)

        xt0 = chain(touch_x(0))
        rzg0 = attention(0)
        mix(0, 0, 0)
        mix(0, 0, 1)
        conv(0, 0, xt0[-1], rzg0)
        mix(0, 1, 0)
        mix(0, 1, 1)
        conv(0, 1, None, rzg0)

        xt1 = chain(touch_x(1), prev=xt0[-1])
        rzg1 = attention(1)
        mix(1, 0, 0)
        mix(1, 0, 1)
        conv(1, 0, xt1[-1], rzg1)
        mix(1, 1, 0)
        mix(1, 1, 1)
        conv(1, 1, None, rzg1)

    nc.compile()
    return nc


def _prep(x, kernels, w1, b1, w2, b2):
    """Host-side marshaling: dtype casts + layout/padding rearrangement only."""
    xp = np.zeros((16, C, PADW, PADW), dtype=ml_dtypes.bfloat16)
    xp[:, :, 1:57, 1:57] = x
    xh = np.ascontiguousarray(xp.reshape(16, C, PADHW))
    # bank[i, ((ot*IT+it)*8+k)*WSL + s*128 + o] = kernels[k, ot*128+o, it*128+i, s]
    kr = kernels.reshape(8, OT, 128, IT, 128, S)   # k, ot, o, it, i, s
    bankh = np.ascontiguousarray(
        kr.transpose(4, 1, 3, 0, 5, 2).reshape(128, OT * IT * 8 * WSL)
        .astype(ml_dtypes.bfloat16))
    consts = np.zeros((128, C_COLS), dtype=np.float32)
    consts[:, C_W1A:C_W1A + 64] = w1[0:128]
    consts[:, C_W1B:C_W1B + 64] = w1[128:256]
    consts[0:64, C_W2:C_W2 + 8] = w2
    consts[0:64, C_B1C] = b1
    consts[0, C_B2:C_B2 + 8] = b2
    consts[0, C_ONES:C_ONES + 128] = 1.0
    consts[0:8, C_ID8:C_ID8 + 8] = np.eye(8, dtype=np.float32)
    bones = np.ones((8, 128), dtype=ml_dtypes.bfloat16)
    return xh, bankh, consts, bones


def kernel(x, kernels, w1, b1, w2, b2):
    global _cached
    if _cached is None:
        _cached = _build()
    nc = _cached
    xh, bankh, consts, bones = _prep(np.asarray(x, dtype=np.float32),
                                     np.asarray(kernels, dtype=np.float32),
                                     np.asarray(w1, dtype=np.float32),
                                     np.asarray(b1, dtype=np.float32),
                                     np.asarray(w2, dtype=np.float32),
                                     np.asarray(b2, dtype=np.float32))
    in_maps = [{"x": xh[c * B:(c + 1) * B], "bank": bankh,
                "consts": consts, "bones": bones} for c in range(N_CORES)]
    res = run_bass_kernel_spmd(nc, in_maps, list(range(N_CORES)))
    out = np.concatenate(
        [res.results[c]["y"].reshape(B, C, H, W_IMG) for c in range(N_CORES)],
        axis=0)
    return out.astype(np.float32)


# revision 6
# speedup vs baseline: 1.1969x; 1.1132x over previous
"""DynamicConvolution Trainium2 kernel (8 NeuronCores, data-parallel over batch).

Reference computation (per sample b):
  pooled = mean(x[b], spatial); h = relu(pooled @ w1 + b1)
  alpha  = softmax(h @ w2 + b2)                   [8]
  W[b]   = sum_k alpha[k] * kernels[k]            [256,256,3,3]
  y[b]   = conv2d(x[b], W[b], pad=1)              [256,56,56]

Sharding: batch 16 -> 2 samples per core; bank + MLP weights replicated.

v4 design (PE does conv only; mixing on VectorE; alpha path on-chip):
  - bank arrives as [i128, (ot, it, k, s, o)] bf16: per (ot,it) block, 8
    contiguous k-slices [128, 1152] in conv-weight (s,o) order.  Kernel
    mixing = 8 chained scalar_tensor_tensor passes on VectorE
    (acc = bank_k * e_k + acc, bf16 ping-pong, last pass writes wsb),
    each pass waiting only on its own k-slice DMA.
  - softmax never touches DRAM: e = exp(h@w2 + b2) unnormalized;
    [1,8]->[8,1] via a tiny PE matmul (lhsT=e, rhs=ones); 1/sum(e) folds
    into the conv evacuation as a per-sample activation scale; 1/HW and
    b1 fold into the relu activation (scale+bias).
  - sample-1 attention is interleaved into sample-0's conv stream: its
    pooling reduces sit between mix emissions on VectorE (they chase the
    late x(b1) DMA), and its tiny PE matmuls hook into conv(0,1) between
    row-blocks so the PE reaches them right as psum2(b1) lands.
  - DMA priority order (inputs on the SP queue, in-order issue): x(b0)
    in 4-col-chunks per i-tile (pooling chases chunks), bank(o0),
    bank(o1), x(b1).  y writes ride the idle GpSimd queue.
  - conv groups in DMA arrival order (b0,o0),(b0,o1),(b1,o0),(b1,o1):
    per (o_t, b, t) PSUM block of 18 accumulating matmuls
    [o128,448] += W[i128,o128]^T @ xpad[i128, 8x56 window]; 7 PSUM bufs
    let the scheduler run it0-tap matmuls of many blocks while it1 is
    still mixing; ScalarE evacuates fp32 (scaled by 1/z); DMA out.

Sync discipline (walrus permits ONE semaphore wait per engine instruction):
  - conv matmul waits split: lhsT dep on InstLdweights (wsb via VectorE
    sem), rhs dep on InstMatmult; PSUM-reuse WAR rides InstMatmult.  The
    x-chunk DMAs are transitively observed by the PE through the
    attention matmul chain (hT waits psum2 which waited every chunk), so
    the first conv matmul of each sample rides that edge via
    add_dep_helper instead of touch matmuls.
  - mix stt ops: single external wait = own bank k-slice DMA; alpha
    scalar + acc are VectorE-local.
  - consts first observed on VectorE by a 1x1 copy; the diag op rides it.
    rzsb first observed on ScalarE by a 1x1 copy; evacs ride it.
  - all tiny MLP PSUM tiles share one bank tag sequentially.
"""

import numpy as np
import ml_dtypes
from contextlib import ExitStack

try:
    import concourse.bass as bass
except ImportError:  # fresh grading dir: repo paths not on sys.path yet
    import sys
    for p in ("/opt/trn_rl_repo", "/root/.axon_site/_ro/trn_rl_repo"):
        if p not in sys.path:
            sys.path.append(p)
    import concourse.bass as bass

import concourse.mybir as mybir
import concourse.tile as tile
from concourse import bacc
from concourse.tile import add_dep_helper
from concourse.bass_utils import run_bass_kernel_spmd

F32 = mybir.dt.float32
BF16 = mybir.dt.bfloat16
AX = mybir.AxisListType.X
RELU = mybir.ActivationFunctionType.Relu
EXP = mybir.ActivationFunctionType.Exp
COPY = mybir.ActivationFunctionType.Copy
MUL = mybir.AluOpType.mult
ADD = mybir.AluOpType.add

N_CORES = 8
B = 2               # samples per core
C = 256             # channels
IT = 2              # 128-channel input tiles
OT = 2              # 128-channel output tiles
H = W_IMG = 56
HW = H * W_IMG      # 3136
PADW = 58
PADHW = PADW * PADW  # 3364
NT = 7              # row blocks per image
TB = 448            # 8 rows x 56 cols per conv psum block
S = 9               # conv taps
WSL = S * 128       # 1152 = per (b,o_t,i_t) weight-slice elems
XCH = 4             # x DMA chunks per (b, i_tile)
XC = PADHW // XCH   # 841 elems per x chunk

# consts layout (fp32 [128, 288])
C_W1A, C_W1B, C_W2, C_B1C, C_B2, C_ONES, C_ID8 = 0, 64, 128, 136, 140, 148, 280
C_COLS = 288

_cached = None


def _build():
    nc = bacc.Bacc()
    xin = nc.declare_dram_parameter("x", [B, C, PADHW], BF16, isOutput=False)
    bankin = nc.declare_dram_parameter("bank", [128, OT * IT * 8 * WSL], BF16,
                                       isOutput=False)
    cin = nc.declare_dram_parameter("consts", [128, C_COLS], F32, isOutput=False)
    bin_ = nc.declare_dram_parameter("bones", [8, 128], BF16, isOutput=False)
    y = nc.declare_dram_parameter("y", [B, C, HW], F32, isOutput=True)

    with tile.TileContext(nc) as tc, ExitStack() as ctx:
        sb = ctx.enter_context(tc.tile_pool(name="sb", bufs=1))
        conv_ps = ctx.enter_context(tc.tile_pool(name="cps", bufs=7, space="PSUM"))
        mlp_ps = ctx.enter_context(tc.tile_pool(name="mps", bufs=1, space="PSUM"))

        xpad = sb.tile([128, B * IT * PADHW], BF16, tag="xpad")
        bank = sb.tile([128, OT * IT * 8 * WSL], BF16, tag="bank")
        wsb = sb.tile([128, B * OT * IT * WSL], BF16, tag="wsb")
        outsb = sb.tile([128, B * OT * NT * TB], F32, tag="outsb")
        consts = sb.tile([128, C_COLS], F32, tag="consts")
        bones = sb.tile([8, 128], BF16, tag="bones")
        acc0 = sb.tile([128, WSL], BF16, tag="acc0")
        acc1 = sb.tile([128, WSL], BF16, tag="acc1")
        acc = [acc0, acc1]
        alphab = sb.tile([128, B * 8], F32, tag="alphab")
        rzsb = sb.tile([128, B], F32, tag="rzsb")

        def xv(b, it):
            base = (b * IT + it) * PADHW
            return xpad[:, base:base + PADHW].rearrange("p (r c) -> p r c", c=PADW)

        # ---------- input DMAs on the SP queue, in priority order ----------
        nc.sync.dma_start(consts[:], cin[:])
        nc.sync.dma_start(bones[:], bin_[:])

        def dma_x(b):
            for it in range(IT):
                base = (b * IT + it) * PADHW
                for cch in range(XCH):
                    nc.sync.dma_start(
                        xpad[:, base + cch * XC: base + (cch + 1) * XC],
                        xin[b, it * 128:(it + 1) * 128, cch * XC:(cch + 1) * XC])

        def dma_bank(ot):
            for it in range(IT):
                for k in range(8):
                    off = ((ot * IT + it) * 8 + k) * WSL
                    nc.sync.dma_start(bank[:, off:off + WSL],
                                      bankin[:, off:off + WSL])

        # ---------- shared small tiles
        ctch = sb.tile([1, 1], F32, tag="ctch")
        rztch = sb.tile([1, 1], F32, tag="rztch")
        partials = sb.tile([128, B * IT * XCH], F32, tag="partials")
        psum2 = sb.tile([128, B * IT], F32, tag="psum2")

        def att_pool(b):
            """channel sums on VectorE, chasing the x(b) chunk DMAs."""
            for it in range(IT):
                base = (b * IT + it) * PADHW
                for cch in range(XCH):
                    j = (b * IT + it) * XCH + cch
                    nc.vector.reduce_sum(
                        partials[:, j:j + 1],
                        xpad[:, base + cch * XC: base + (cch + 1) * XC], axis=AX)
            for it in range(IT):
                j = b * IT + it
                nc.vector.reduce_sum(psum2[:, j:j + 1],
                                     partials[:, j * XCH:(j + 1) * XCH], axis=AX)

        def att_mlp(b):
            """MLP -> e=exp(scores) -> alphab[:, b*8:(b+1)*8], z/rz.
            Returns (first PE matmul, rz tile) for dep threading."""
            hT_ps = mlp_ps.tile([64, 1], F32, tag="mlp")
            h1 = nc.tensor.matmul(hT_ps[:], consts[:, C_W1A:C_W1A + 64],
                                  psum2[:, b * IT: b * IT + 1],
                                  start=True, stop=False)
            nc.tensor.matmul(hT_ps[:], consts[:, C_W1B:C_W1B + 64],
                             psum2[:, b * IT + 1: b * IT + 2],
                             start=False, stop=True)
            hT = sb.tile([64, 1], F32, tag=f"hTs{b}")
            nc.scalar.activation(hT[:], hT_ps[:], RELU,
                                 bias=consts[0:64, C_B1C:C_B1C + 1],
                                 scale=1.0 / HW)

            sc_ps = mlp_ps.tile([1, 8], F32, tag="mlp")
            nc.tensor.matmul(sc_ps[:], consts[0:1, C_ONES:C_ONES + 1],
                             consts[0:1, C_B2:C_B2 + 8], start=True, stop=False)
            nc.tensor.matmul(sc_ps[:], hT[:], consts[0:64, C_W2:C_W2 + 8],
                             start=False, stop=True)

            # e = exp(scores); scores ~ +-0.1 so no max-subtraction needed
            ex = sb.tile([1, 8], F32, tag=f"ex{b}")
            nc.scalar.activation(ex[:], sc_ps[:], EXP)

            # e as a column via PE transpose, then diag(e) and broadcast
            e_ps = mlp_ps.tile([8, 1], F32, tag="mlp")
            nc.tensor.matmul(e_ps[:], ex[:], consts[0:1, C_ONES:C_ONES + 1],
                             start=True, stop=True)
            e8 = sb.tile([8, 1], F32, tag=f"e8{b}")
            nc.vector.tensor_copy(e8[:], e_ps[:])
            diag8 = sb.tile([8, 8], BF16, tag=f"dg{b}")
            dg_i = nc.vector.tensor_scalar_mul(
                diag8[:], consts[0:8, C_ID8:C_ID8 + 8], e8[:, 0:1])
            if b == 0:
                ctch_i = nc.vector.tensor_copy(ctch[:], consts[0:1, 0:1])
                add_dep_helper(dg_i.ins, ctch_i.ins, sync=False,
                               reason="consts seen on DVE")
            aT_ps = mlp_ps.tile([128, 8], F32, tag="mlp")
            nc.tensor.matmul(aT_ps[:], bones[:], diag8[:], start=True, stop=True)
            nc.vector.tensor_copy(alphab[:, b * 8:(b + 1) * 8], aT_ps[:])

            z = sb.tile([1, 1], F32, tag=f"z{b}")
            nc.vector.reduce_sum(z[:], ex[:], axis=AX)
            rz = sb.tile([1, 1], F32, tag=f"rz{b}")
            nc.vector.reciprocal(rz[:], z[:])
            return h1, rz

        def att_rz(b, rz):
            """rzsb[:, b] = 1/z broadcast to 128 partitions (evac scale)."""
            rz_ps = mlp_ps.tile([128, 1], F32, tag="mlp")
            nc.tensor.matmul(rz_ps[:], consts[0:1, C_ONES:C_ONES + 128],
                             rz[:], start=True, stop=True)
            nc.vector.tensor_copy(rzsb[:, b:b + 1], rz_ps[:])
            return nc.scalar.activation(rztch[:], rzsb[0:1, b:b + 1], COPY)

        def mix(b, ot, it):
            """wsb slice (b,ot,it) = sum_k e[b,k] * bank(ot,it,k) on DVE."""
            woff = ((b * OT + ot) * IT + it) * WSL
            blk = (ot * IT + it) * 8
            a = alphab[:, b * 8:(b + 1) * 8]
            nc.vector.tensor_scalar_mul(
                acc[0][:], bank[:, blk * WSL:(blk + 1) * WSL], a[:, 0:1])
            for k in range(1, 8):
                out = wsb[:, woff:woff + WSL] if k == 7 else acc[k % 2][:]
                nc.vector.scalar_tensor_tensor(
                    out, bank[:, (blk + k) * WSL:(blk + k + 1) * WSL],
                    a[:, k:k + 1], acc[1 - k % 2][:], MUL, ADD)

        def conv(b, ot, xobs, rzgate, hooks=None):
            """One (sample, o-tile) conv group: NT blocks of 18 matmuls.
            hooks: {t: callable} emitted after block t (PE stream placement).
            Returns rzgate (possibly produced by a hook)."""
            gi = b * OT + ot
            for t in range(NT):
                ps = conv_ps.tile([128, TB], F32, tag="convps")
                n_mm = 0
                for it in range(IT):
                    woff = ((b * OT + ot) * IT + it) * WSL
                    v = xv(b, it)
                    for s in range(S):
                        kh, kw = s // 3, s % 3
                        mm = nc.tensor.matmul(
                            ps[:],
                            wsb[:, woff + s * 128: woff + (s + 1) * 128],
                            v[:, 8 * t + kh: 8 * t + kh + 8, kw:kw + 56],
                            start=(n_mm == 0), stop=(n_mm == 17))
                        if n_mm == 0 and t == 0 and xobs is not None:
                            add_dep_helper(mm.ins, xobs.ins, sync=False,
                                           reason="xpad observed via attention")
                        n_mm += 1
                blk = gi * NT + t
                ev = nc.scalar.activation(outsb[:, blk * TB:(blk + 1) * TB],
                                          ps[:], COPY,
                                          scale=rzsb[:, b:b + 1])
                if t == 0:
                    add_dep_helper(ev.ins, rzgate.ins, sync=False,
                                   reason="rzsb seen on ACT")
                nc.gpsimd.dma_start(
                    y[b, ot * 128:(ot + 1) * 128, t * TB:(t + 1) * TB],
                    outsb[:, blk * TB:(blk + 1) * TB])
                if hooks and t in hooks:
                    hooks[t]()

        # ---------- emission in intended runtime order ----------
        dma_x(0)
        dma_bank(0)
        dma_bank(1)
        dma_x(1)

        att_pool(0)
        h1_0, rz0 = att_mlp(0)
        rzg0 = att_rz(0, rz0)
        mix(0, 0, 0)
        mix(0, 0, 1)
        conv(0, 0, h1_0, rzg0)
        mix(0, 1, 0)
        mix(0, 1, 1)

        # sample-1 attention interleaves with conv(0,1):
        att_pool(1)
        state = {}

        def hook_mlp1():
            state["h1_1"], state["rz1"] = att_mlp(1)

        def hook_rz1():
            state["rzg1"] = att_rz(1, state["rz1"])

        conv(0, 1, None, rzg0, hooks={2: hook_mlp1, 4: hook_rz1})

        mix(1, 0, 0)
        mix(1, 0, 1)
        conv(1, 0, state["h1_1"], state["rzg1"])
        mix(1, 1, 0)
        mix(1, 1, 1)
        conv(1, 1, None, state["rzg1"])

    nc.compile()
    return nc


def _prep(x, kernels, w1, b1, w2, b2):
    """Host-side marshaling: dtype casts + layout/padding rearrangement only."""
    xp = np.zeros((16, C, PADW, PADW), dtype=ml_dtypes.bfloat16)
    xp[:, :, 1:57, 1:57] = x
    xh = np.ascontiguousarray(xp.reshape(16, C, PADHW))
    # bank[i, ((ot*IT+it)*8+k)*WSL + s*128 + o] = kernels[k, ot*128+o, it*128+i, s]
    kr = kernels.reshape(8, OT, 128, IT, 128, S)   # k, ot, o, it, i, s
    bankh = np.ascontiguousarray(
        kr.transpose(4, 1, 3, 0, 5, 2).reshape(128, OT * IT * 8 * WSL)
        .astype(ml_dtypes.bfloat16))
    consts = np.zeros((128, C_COLS), dtype=np.float32)
    consts[:, C_W1A:C_W1A + 64] = w1[0:128]
    consts[:, C_W1B:C_W1B + 64] = w1[128:256]
    consts[0:64, C_W2:C_W2 + 8] = w2
    consts[0:64, C_B1C] = b1
    consts[0, C_B2:C_B2 + 8] = b2
    consts[0, C_ONES:C_ONES + 128] = 1.0
    consts[0:8, C_ID8:C_ID8 + 8] = np.eye(8, dtype=np.float32)
    bones = np.ones((8, 128), dtype=ml_dtypes.bfloat16)
    return xh, bankh, consts, bones


def kernel(x, kernels, w1, b1, w2, b2):
    global _cached
    if _cached is None:
        _cached = _build()
    nc = _cached
    xh, bankh, consts, bones = _prep(np.asarray(x, dtype=np.float32),
                                     np.asarray(kernels, dtype=np.float32),
                                     np.asarray(w1, dtype=np.float32),
                                     np.asarray(b1, dtype=np.float32),
                                     np.asarray(w2, dtype=np.float32),
                                     np.asarray(b2, dtype=np.float32))
    in_maps = [{"x": xh[c * B:(c + 1) * B], "bank": bankh,
                "consts": consts, "bones": bones} for c in range(N_CORES)]
    res = run_bass_kernel_spmd(nc, in_maps, list(range(N_CORES)))
    out = np.concatenate(
        [res.results[c]["y"].reshape(B, C, H, W_IMG) for c in range(N_CORES)],
        axis=0)
    return out.astype(np.float32)
